# revision 2
# baseline (speedup 1.0000x reference)
"""Trainium2 Bass kernel v2 for nn_EncoderDecoderTransformer (sparse kNN enc attn).

Changes vs v1 (2308us baseline):
  - Pair exchange carries the LN1 output h (0.5MB bf16 -> 1MB AllGather, ~40us)
    instead of K/V (1MB -> 2MB, ~67us); each core projects the peer's K/V
    locally from the received h. The AG fires right after LN1, covered by the
    own-half K/V/Q projections and own-half attention. The peer half of the AG
    output is read with a runtime-register (ds) dynamic DMA offset, so one SPMD
    program serves both pair members. (remote_dma_broadcast was tried and wedges
    the device under this PJRT runtime.)
  - Attention restructured own-half-first: key tiles 0-3 are the core's own 512
    tokens (K/V straight from local projection output in SBUF), tiles 4-7 the
    pair-peer's (from the rx buffer). All mask inputs are permuted own-first
    per core.
  - Causal attention: own half is lower-triangular by construction -> scores and
    AV matmuls and exp column-trimmed to q >= 128*c, single [128,128] triangular
    multiply on the diagonal block; peer half masked via exp bias (+0 on odd
    cores, -1e30 on even) -> no mask multiplies at all for the peer half.
  - kNN mask applied once per (hp, kt) on a j-paired [128, 2, 512] e-tile with a
    stride-0-replicated mask AP.
  - Cross-attention: single enc-out exchange (ebx) after the encoder; per layer
    K/V projected locally over all 1024 enc tokens (own from ebx, peer from
    rx_eob) in the same kvx layout.
  - LayerNorm h = x*a + c computed in bf16 (2x DVE mode); a/c broadcasts evicted
    bf16.
  - K-projection bias applied as per-partition column add on eviction (no rank-1
    bias matmuls for K).
  - Softmax denominators: reciprocal straight from the PSUM ones-row.
  - No pe_warm filler matmuls; no DRAM K/V bounce.
"""

import os
import numpy as np
import ml_dtypes

BF16 = ml_dtypes.bfloat16

D, F, H, NE, ND, KNN = 512, 2048, 8, 4, 4, 16
B, LE, LD = 4, 1024, 1024
DH = D // H
NCORE = 8
P = 128
TOWN = 512
NDT = D // P
NKT = LE // P
NEG = -1e30
EPS = 1e-5
KV_K = NDT * TOWN            # 2048 cols: K^T feature-major [4,512]
KV_V = NDT * H * 65          # 2080 cols: V token-major [4,8,65] incl ones col
KV_W = KV_K + KV_V           # 4128

_CACHE = {}


def build(n_enc=NE, n_dec=ND, ncore=NCORE):
    from contextlib import ExitStack

    import concourse.bacc as bacc
    import concourse.tile as tile
    import concourse.mybir as mybir
    import concourse.bass as cbass

    f32 = mybir.dt.float32
    bf16 = mybir.dt.bfloat16
    AF = mybir.ActivationFunctionType
    OP = mybir.AluOpType

    nc = bacc.Bacc("TRN2", target_bir_lowering=False, debug=False, num_devices=ncore)

    def din(name, shape, dt=f32):
        return nc.dram_tensor(name, shape, dt, kind="ExternalInput")

    x0T = din("x0T", [NDT, P, TOWN])
    y0T = din("y0T", [NDT, P, TOWN])
    xq2_d = din("xq2", [TOWN, 3])        # 2*xyz own queries (global order)
    xq2row_d = din("xq2row", [3, TOWN])
    xkn_d = din("xkn", [LE, 4])          # [xyz, |xyz|^2] keys, OWN-FIRST order
    xrow_d = din("xrow", [4, LE])        # same, transposed
    bosrow = din("bosrow", [1, TOWN])
    boskey_d = din("boskey", [P, NKT])  # 1.0 where global token 0 lives
    tri_d = din("tri", [P, P], bf16)          # causal diag block: k<=q allow
    pbias_d = din("pbias", [P, 1])            # causal peer-half exp bias

    ew_qkv = din("ew_qkv", [NE, D, 3 * D], bf16)
    ew_out = din("ew_out", [NE, D, D], bf16)
    ew_f1 = din("ew_f1", [NE, D, F], bf16)
    ew_f2 = din("ew_f2", [NE, F, D], bf16)
    eb_qkv = din("eb_qkv", [NE, 3 * D, 1])
    eb_out = din("eb_out", [NE, D, 1])
    eb_f1 = din("eb_f1", [NE, F, 1])
    eb_f2 = din("eb_f2", [NE, D, 1])

    dw_saqkv = din("dw_saqkv", [ND, D, 3 * D], bf16)
    db_saqkv = din("db_saqkv", [ND, 3 * D, 1])
    dw_saout = din("dw_saout", [ND, D, D], bf16)
    db_saout = din("db_saout", [ND, D, 1])
    dw_caqkv = din("dw_caqkv", [ND, D, 3 * D], bf16)
    db_caqkv = din("db_caqkv", [ND, 3 * D, 1])
    dw_caout = din("dw_caout", [ND, D, D], bf16)
    db_caout = din("db_caout", [ND, D, 1])
    dw_f1 = din("dw_f1", [ND, D, F], bf16)
    db_f1 = din("db_f1", [ND, F, 1])
    dw_f2 = din("dw_f2", [ND, F, D], bf16)
    db_f2 = din("db_f2", [ND, D, 1])
    eb_qkv_bf = din("eb_qkv_bf", [NE, 3 * D, 1], bf16)
    db_saqkv_bf = din("db_saqkv_bf", [ND, 3 * D, 1], bf16)
    db_caqkv_bf = din("db_caqkv_bf", [ND, 3 * D, 1], bf16)

    peeridx_d = din("peeridx", [1, 1], mybir.dt.uint32)

    enc_part = nc.dram_tensor("enc_part", [NDT, P, TOWN], f32, kind="ExternalOutput")
    dec_part = nc.dram_tensor("dec_part", [NDT, P, TOWN], f32, kind="ExternalOutput")
    DUMP = bool(os.environ.get("KQ_DUMP"))
    if DUMP:
        bf16_ = mybir.dt.bfloat16
        d_hx = nc.dram_tensor("d_hx", [P, NDT * TOWN], bf16_, kind="ExternalOutput")
        d_rxh = nc.dram_tensor("d_rxh", [P, NDT * TOWN], bf16_, kind="ExternalOutput")
        d_kvx = nc.dram_tensor("d_kvx", [P, KV_W], bf16_, kind="ExternalOutput")
        d_kvp = nc.dram_tensor("d_kvp", [P, KV_W], bf16_, kind="ExternalOutput")
        d_allow = nc.dram_tensor("d_allow", [NKT, P, TOWN], bf16_, kind="ExternalOutput")
        d_x1 = nc.dram_tensor("d_x1", [NDT, P, TOWN], f32, kind="ExternalOutput")
    dump_done = [False]

    PAIRS = [list(p) for p in zip(range(0, ncore, 2), range(1, ncore, 2))]
    exch_n = [0]

    with tile.TileContext(nc) as tc, ExitStack() as ctx:
        ep = ctx.enter_context

        pc = ep(tc.tile_pool(name="pc", bufs=1))
        p_allow = ep(tc.tile_pool(name="p_allow", bufs=8))
        ps_s = ep(tc.tile_pool(name="ps_s", bufs=3, space="PSUM"))
        ps_o = ep(tc.tile_pool(name="ps_o", bufs=3, space="PSUM"))
        ps_mm = ep(tc.tile_pool(name="ps_mm", bufs=2, space="PSUM"))
        p_dram = ep(tc.tile_pool(name="p_dram", bufs=2, space="DRAM"))

        # ---- constants ----
        ones_sq_bf = pc.tile([P, P], bf16)
        nc.vector.memset(ones_sq_bf, 1.0)
        ones_row = pc.tile([1, P], f32)
        nc.vector.memset(ones_row, 1.0)
        ones_row_bf = pc.tile([1, P], bf16)
        nc.vector.memset(ones_row_bf, 1.0)
        eps_sb = pc.tile([P, 1], f32)
        nc.vector.memset(eps_sb, EPS)

        boskey_sb = pc.tile([P, NKT], f32)
        nc.sync.dma_start(out=boskey_sb, in_=boskey_d[:, :])
        tri_sb = pc.tile([P, P], bf16)
        nc.sync.dma_start(out=tri_sb, in_=tri_d[:, :])
        pbias_sb = pc.tile([P, 1], f32)
        nc.sync.dma_start(out=pbias_sb, in_=pbias_d[:, :])

        def rep2(ap, n=2):
            # [P, N] AP -> [P, n, N] with stride-0 middle dim
            return cbass.AP(
                tensor=ap.tensor, offset=ap.offset,
                ap=[list(ap.ap[0]), [0, n]] + [list(a) for a in ap.ap[1:]],
            )

        def build_mask():
            allow_sb = []
            with tc.tile_pool(name="p_mask", bufs=2) as p_mask, \
                 tc.tile_pool(name="p_mbc", bufs=1) as p_mbc, \
                 tc.tile_pool(name="p_m8", bufs=8) as p_m8:
                def bcast_rows(dram_row_ap, pool, n_free, tag):
                    t = pool.tile([P, n_free], f32, tag=tag, name=tag)
                    src_ap = cbass.AP(
                        tensor=dram_row_ap.tensor, offset=dram_row_ap.offset,
                        ap=[[0, P]] + list(dram_row_ap.ap),
                    )
                    nc.sync.dma_start(out=t, in_=src_ap)
                    return t

                bos_sb = p_mbc.tile([1, TOWN], f32, tag="bos", name="bos")
                nc.sync.dma_start(out=bos_sb, in_=bosrow[:, :])
                tcol_dram = p_dram.tile([4, P, 1], f32, tag="tcol")
                for qt in range(4):
                    bcx = []
                    for c in range(4):
                        t = bcast_rows(xrow_d[c], p_mask, LE, tag="bcx")
                        bcx.append(t)
                    xqc = p_m8.tile([P, 3], f32, tag="xqc")
                    nc.sync.dma_start(out=xqc, in_=xq2_d[qt * P:(qt + 1) * P, :])
                    s0 = p_mask.tile([P, LE], f32, tag="s")
                    nc.vector.tensor_scalar(s0, bcx[0], xqc[:, 0:1], None, op0=OP.mult)
                    s1 = p_mask.tile([P, LE], f32, tag="s")
                    nc.vector.scalar_tensor_tensor(s1, bcx[1], xqc[:, 1:2], s0, OP.mult, OP.add)
                    s2 = p_mask.tile([P, LE], f32, tag="s")
                    nc.vector.scalar_tensor_tensor(s2, bcx[2], xqc[:, 2:3], s1, OP.mult, OP.add)
                    s3 = p_mask.tile([P, LE], f32, tag="s")
                    nc.vector.tensor_tensor(s3, s2, bcx[3], OP.subtract)
                    m8 = p_m8.tile([P, 8], f32, tag="m8")
                    nc.vector.max(m8, s3)
                    s4 = p_mask.tile([P, LE], f32, tag="s")
                    nc.vector.match_replace(s4, m8, s3, NEG)
                    m8b = p_m8.tile([P, 8], f32, tag="m8")
                    nc.vector.max(m8b, s4)
                    s5 = p_mask.tile([P, LE], f32, tag="s")
                    nc.vector.match_replace(s5, m8b, s4, NEG)
                    m8c = p_m8.tile([P, 8], f32, tag="m8")
                    nc.vector.max(m8c, s5)
                    nc.sync.dma_start(out=tcol_dram[qt], in_=m8c[:, 0:1])
                t_row = p_mbc.tile([1, TOWN], f32, tag="t_row", name="t_row")
                nc.sync.dma_start(
                    out=t_row, in_=tcol_dram.rearrange("a p one -> one (a p)")
                )
                t2 = p_mbc.tile([1, TOWN], f32, tag="t2", name="t2")
                nc.vector.tensor_tensor(t2, t_row, bos_sb, OP.min)
                t2_dram = p_dram.tile([1, TOWN], f32, tag="t2d")
                nc.sync.dma_start(out=t2_dram, in_=t2)
                t_bc = bcast_rows(t2_dram[0], p_mbc, TOWN, tag="t_bc")
                bq = []
                for c in range(3):
                    t = bcast_rows(xq2row_d[c], p_mbc, TOWN, tag=f"bq{c}")
                    bq.append(t)
                for kt in range(NKT):
                    xkc = p_m8.tile([P, 4], f32, tag="xkc")
                    nc.sync.dma_start(out=xkc, in_=xkn_d[kt * P:(kt + 1) * P, :])
                    u0 = p_mask.tile([P, TOWN], f32, tag="st")
                    nc.vector.tensor_scalar(u0, bq[0], xkc[:, 0:1], None, op0=OP.mult)
                    u1 = p_mask.tile([P, TOWN], f32, tag="st")
                    nc.vector.scalar_tensor_tensor(u1, bq[1], xkc[:, 1:2], u0, OP.mult, OP.add)
                    u2 = p_mask.tile([P, TOWN], f32, tag="st")
                    nc.vector.scalar_tensor_tensor(u2, bq[2], xkc[:, 2:3], u1, OP.mult, OP.add)
                    u3 = p_mask.tile([P, TOWN], f32, tag="st")
                    nc.vector.tensor_scalar(u3, u2, xkc[:, 3:4], None, op0=OP.subtract)
                    al = p_allow.tile([P, TOWN], bf16, tag="allow")
                    nc.vector.tensor_tensor(al, u3, t_bc, OP.is_ge)
                    nc.vector.tensor_scalar(al, al, boskey_sb[:, kt:kt + 1], None, op0=OP.max)
                    allow_sb.append(al)
            return allow_sb

        allow_sb = []

        def build_mask_hook():
            allow_sb.extend(build_mask())

        # ================= helpers =================
        def load_w(pool, dram_ap, kchunks, cols, tag):
            t = pool.tile([P, kchunks, cols], bf16, tag=tag, name=tag)
            nc.sync.dma_start(
                out=t, in_=dram_ap.rearrange("(kc p) m -> p kc m", p=P)
            )
            return t

        def layer_norm(xs, out_dt, out_pool, out_tag, out_aps=None):
            # stats broadcast across all partitions via ones[128,128] lhsT: the
            # mean/var/rstd pipeline runs at [128, 512] and a/c drop out of the
            # Scalar/Vector engines already broadcast -- no extra PE round trip
            sqs = []
            xbs = []
            for dt in range(NDT):
                xb = p_lnsq.tile([P, TOWN], bf16, tag="lnxb")
                nc.vector.tensor_copy(xb, xs[dt])
                xbs.append(xb)
                sq = p_lnsq.tile([P, TOWN], bf16, tag="lnsq")
                nc.scalar.activation(sq, xbs[dt], AF.Square)
                sqs.append(sq)
            ps_mean = ps_mm.tile([P, TOWN], f32, tag="mm")
            for dt in range(NDT):
                nc.tensor.matmul(ps_mean, ones_sq_bf, xbs[dt], start=dt == 0, stop=dt == 3)
            ps_sq = ps_mm.tile([P, TOWN], f32, tag="mm")
            for dt in range(NDT):
                nc.tensor.matmul(ps_sq, ones_sq_bf, sqs[dt], start=dt == 0, stop=dt == 3)
            mu = p_lnmu.tile([P, TOWN], f32, tag="lnmu")
            nc.vector.tensor_single_scalar(mu, ps_mean, 1.0 / D, OP.mult)
            musq = p_lnw.tile([P, TOWN], f32, tag="lntmp")
            nc.vector.tensor_tensor(musq, mu, mu, OP.mult)
            var = p_lnw.tile([P, TOWN], f32, tag="lntmp")
            nc.vector.scalar_tensor_tensor(var, ps_sq, 1.0 / D, musq, OP.mult, OP.subtract)
            lnv = p_lnw.tile([P, TOWN], f32, tag="lntmp")
            nc.scalar.activation(lnv, var, AF.Ln, bias=eps_sb)
            a_sb = p_lnac.tile([P, TOWN], bf16, tag="lna")
            nc.scalar.activation(a_sb, lnv, AF.Exp, scale=-0.5)
            c_sb = p_lnac.tile([P, TOWN], bf16, tag="lnc")
            nc.vector.scalar_tensor_tensor(c_sb, mu, -1.0, a_sb, OP.mult, OP.mult)
            hs = []
            for dt in range(NDT):
                if out_aps is not None:
                    h = out_aps[dt]
                else:
                    h = out_pool.tile([P, TOWN], out_dt, tag=out_tag, name=out_tag)
                nc.vector.tensor_tensor(h, xbs[dt], a_sb, OP.mult)
                nc.vector.tensor_tensor(h, h, c_sb, OP.add)
                hs.append(h)
            return hs

        def proj_fm(w_sb, col_off, n_m, rhs, bias_ap, out_pool, out_tag, out_dt=bf16):
            outs = []
            nk = len(rhs)
            for m in range(n_m):
                ps = ps_mm.tile([P, TOWN], f32, tag="mm")
                for kc in range(nk):
                    nc.tensor.matmul(
                        ps, w_sb[:, kc, col_off + m * P:col_off + (m + 1) * P],
                        rhs[kc], start=kc == 0, stop=kc == nk - 1,
                    )
                bcol = p_bias.tile([P, 1], f32, tag="bcol")
                nc.sync.dma_start(out=bcol, in_=bias_ap[col_off + m * P:col_off + (m + 1) * P, :])
                o = out_pool.tile([P, TOWN], out_dt, tag=out_tag, name=out_tag)
                nc.vector.tensor_scalar(o, ps, bcol, None, op0=OP.add)
                outs.append(o)
            return outs

        def kv_views(t):
            kv = t[:, 0:KV_K].rearrange("p (m c) -> p m c", m=NDT)
            vv = t[:, KV_K:KV_W].rearrange("p (t h c) -> p t h c", t=NDT, h=H)
            return kv, vv

        def proj_kv(w_sb, k_woff, v_woff, k_boff, v_boff, rhs, bias_ap,
                    bias_bf_ap, kvx):
            """K (feature-major, bcol bias on eviction) + V (token-major,
            bias folded into the downstream out-proj bias host-side)."""
            kv, vv = kv_views(kvx)
            for m in range(NDT):
                ps = ps_mm.tile([P, TOWN], f32, tag="mm")
                for kc in range(NDT):
                    nc.tensor.matmul(
                        ps, w_sb[:, kc, k_woff + m * P:k_woff + (m + 1) * P],
                        rhs[kc], start=kc == 0, stop=kc == NDT - 1,
                    )
                bcol = p_bias.tile([P, 1], f32, tag="bcol")
                nc.sync.dma_start(out=bcol, in_=bias_ap[k_boff + m * P:k_boff + (m + 1) * P, :])
                nc.vector.tensor_scalar(kv[:, m, :], ps, bcol, None, op0=OP.add)
            for m in range(NDT):
                ps = ps_mm.tile([P, TOWN], f32, tag="mm")
                for kc in range(NDT):
                    nc.tensor.matmul(
                        ps, rhs[kc][:, m * P:(m + 1) * P],
                        w_sb[:, kc, v_woff:v_woff + D],
                        start=kc == 0, stop=kc == NDT - 1,
                    )
                nc.vector.tensor_copy(
                    vv[:, m, :, 0:64],
                    ps[:, :].rearrange("p (h c) -> p h c", h=H),
                )
                nc.vector.memset(vv[:, m, :, 64:65], 1.0)

        pidx_sb = pc.tile([1, 1], mybir.dt.uint32)
        nc.sync.dma_start(out=pidx_sb, in_=peeridx_d[:, :])
        peer_reg_raw = nc.sync.alloc_register("peeridx_reg")
        nc.sync.reg_load(peer_reg_raw, pidx_sb[0:1, 0:1])
        peer_reg = nc.sync.snap(peer_reg_raw, donate=True, min_val=0, max_val=1)

        LOCAL_DBG = bool(os.environ.get("KQ_LOCAL"))

        def exchange_fire(tx_ap, w, dt_=bf16):
            """DMA tx (SBUF, [P, w]) to DRAM and fire the pair AllGather.
            Returns the [2, P, w] gathered DRAM buffer."""
            exch_n[0] += 1
            g = exch_n[0]
            if LOCAL_DBG:
                bin_ = p_dram.tile([P, w], dt_, tag=f"xin{g}", name=f"xin{g}")
                nc.sync.dma_start(out=bin_, in_=tx_ap)
                return bin_
            bin_ = p_dram.tile([P, w], dt_, tag=f"xin{g}", name=f"xin{g}")
            bout = p_dram.tile([2, P, w], dt_, tag=f"xout{g}", name=f"xout{g}")
            nc.sync.dma_start(out=bin_, in_=tx_ap)
            nc.gpsimd.collective_compute(
                "AllGather", OP.bypass, replica_groups=PAIRS,
                ins=[bin_[:].opt()], outs=[bout[:].opt()],
            )
            return bout

        def exchange_recv(bout, rx_ap):
            if LOCAL_DBG:
                nc.sync.dma_start(out=rx_ap, in_=bout[:, :])
                return
            # Bounce through a static DRAM->DRAM copy first: a dynamic (SWDGE)
            # DMA reading the collective's output directly does not reliably
            # order after the collective on HW; static-after-collective and
            # dynamic-after-static are both proven paths.
            g = exch_n[0]
            w = bout.shape[2]
            bout2 = p_dram.tile(list(bout.shape), bout.dtype,
                                tag=f"xmid{g}", name=f"xmid{g}")
            nc.sync.dma_start(out=bout2[:, :, :], in_=bout[:, :, :])
            nc.sync.dma_start(
                out=rx_ap,
                in_=bout2[cbass.ds(peer_reg, 1)].rearrange("o p w -> (o p) w"),
            )

        def attention(Qs, srcs, mode):
            # srcs: (own_tile, peer_tile) in kvx layout; mode: 'knn'|'causal'|None
            views = [kv_views(t) for t in srcs]
            OTs = []
            for hp in range(4):
                psO = []
                for _j in range(2):
                    psO_t = ps_o.tile([65, TOWN], f32, tag="pso")
                    psO.append(psO_t)
                for si, (kv, vv) in enumerate(views):
                    for c in range(NDT):
                        slot = si * NDT + c
                        c0 = c * P if (mode == "causal" and si == 0) else 0
                        e = p_e.tile([P, 2, TOWN], bf16, tag="e")
                        for j in range(2):
                            rows = slice(j * 64, (j + 1) * 64)
                            psS = ps_s.tile([P, TOWN], f32, tag="pss")
                            nc.tensor.matmul(
                                psS[:, c0:], kv[rows, hp, c * P:(c + 1) * P],
                                Qs[hp][rows, c0:], start=True, stop=True,
                            )
                            if mode == "causal" and si == 1:
                                nc.scalar.activation(
                                    e[:, j, :], psS, AF.Exp, bias=pbias_sb, scale=0.125
                                )
                            else:
                                nc.scalar.activation(
                                    e[:, j, c0:], psS[:, c0:], AF.Exp, scale=0.125
                                )
                        if mode == "knn":
                            for j in range(2):
                                ej = e[:, j, :]
                                nc.vector.tensor_tensor(
                                    ej, ej, allow_sb[slot][:, :], OP.mult
                                )
                        elif mode == "causal" and si == 0:
                            for j in range(2):
                                dg = e[:, j, c * P:(c + 1) * P]
                                nc.vector.tensor_tensor(
                                    dg, dg, tri_sb[:, :], OP.mult
                                )
                        for j in range(2):
                            head = 2 * hp + j
                            nc.tensor.matmul(
                                psO[j][:, c0:], vv[:, c, head, :], e[:, j, c0:],
                                start=slot == 0, stop=slot == 2 * NDT - 1,
                            )
                ot = p_ot.tile([P, TOWN], bf16, tag="ot")
                for j in range(2):
                    den = p_small.tile([1, TOWN], f32, tag="sm")
                    nc.vector.tensor_copy(den, psO[j][64:65, :])
                    rec = p_small.tile([1, TOWN], f32, tag="sm")
                    nc.vector.reciprocal_approx_fast(rec, den)
                    psB = ps_mm.tile([64, TOWN], f32, tag="mm")
                    nc.tensor.matmul(psB, ones_row[:, 0:64], rec, start=True, stop=True)
                    bc = p_bc.tile([64, TOWN], f32, tag="bc")
                    nc.vector.tensor_copy(bc, psB)
                    nc.vector.tensor_tensor(ot[j * 64:(j + 1) * 64, :], psO[j][0:64, :], bc, OP.mult)
                OTs.append(ot)
            return OTs

        def proj_residual(w_sb, col_off, n_k, rhs, bias_ap, xs):
            nxs = []
            for m in range(NDT):
                ps = ps_mm.tile([P, TOWN], f32, tag="mm")
                for kc in range(n_k):
                    nc.tensor.matmul(
                        ps, w_sb[:, kc, col_off + m * P:col_off + (m + 1) * P],
                        rhs[kc], start=kc == 0, stop=kc == n_k - 1,
                    )
                bcol = p_bias.tile([P, 1], f32, tag="bcol")
                nc.sync.dma_start(out=bcol, in_=bias_ap[m * P:(m + 1) * P, :])
                nx = p_x.tile([P, TOWN], f32, tag="x")
                nc.vector.scalar_tensor_tensor(nx, ps, bcol, xs[m], OP.add, OP.add)
                nxs.append(nx)
            return nxs

        def ffn(w1_ap, w2_ap, b1_ap, b2_ap, hs, xs):
            gs = []
            for m in range(F // P):
                w1m = p_w1.tile([P, NDT, P], bf16, tag="wf1")
                nc.sync.dma_start(
                    out=w1m,
                    in_=w1_ap[:, m * P:(m + 1) * P].rearrange("(kc p) m -> p kc m", p=P),
                )
                ps = ps_mm.tile([P, TOWN], f32, tag="mm")
                for kc in range(NDT):
                    nc.tensor.matmul(
                        ps, w1m[:, kc, :], hs[kc],
                        start=kc == 0, stop=kc == NDT - 1,
                    )
                bcol = p_bias.tile([P, 1], f32, tag="bcol")
                nc.sync.dma_start(out=bcol, in_=b1_ap[m * P:(m + 1) * P, :])
                g = p_g.tile([P, TOWN], bf16, tag="g")
                nc.scalar.activation(
                    g, ps, AF.Tanh if os.environ.get("KQ_DBG_TANH") else AF.Gelu,
                    bias=bcol)
                gs.append(g)
            nxs = []
            for m in range(NDT):
                w2m = p_w2.tile([P, F // P, P], bf16, tag="wf2")
                nc.sync.dma_start(
                    out=w2m,
                    in_=w2_ap[:, m * P:(m + 1) * P].rearrange("(kc p) c -> p kc c", p=P),
                )
                ps2 = ps_mm.tile([P, TOWN], f32, tag="mm")
                for kc in range(F // P):
                    nc.tensor.matmul(
                        ps2, w2m[:, kc, :], gs[kc],
                        start=kc == 0, stop=kc == F // P - 1,
                    )
                bcol = p_bias.tile([P, 1], f32, tag="bcol")
                nc.sync.dma_start(out=bcol, in_=b2_ap[m * P:(m + 1) * P, :])
                nx = p_x.tile([P, TOWN], f32, tag="x")
                nc.vector.scalar_tensor_tensor(nx, ps2, bcol, xs[m], OP.add, OP.add)
                nxs.append(nx)
            return nxs

        p_x = ep(tc.tile_pool(name="p_x", bufs=5))
        p_h = ep(tc.tile_pool(name="p_h", bufs=6))
        p_q = ep(tc.tile_pool(name="p_q", bufs=4))
        p_ot = ep(tc.tile_pool(name="p_ot", bufs=4))
        p_e = ep(tc.tile_pool(name="p_e", bufs=2))
        p_lnsq = ep(tc.tile_pool(name="p_lnsq", bufs=4))
        p_lnac = ep(tc.tile_pool(name="p_lnac", bufs=2))
        p_lnw = ep(tc.tile_pool(name="p_lnw", bufs=2))
        p_lnmu = ep(tc.tile_pool(name="p_lnmu", bufs=1))
        p_bc = ep(tc.tile_pool(name="p_bc", bufs=2))
        p_small = ep(tc.tile_pool(name="p_small", bufs=5))
        p_bias = ep(tc.tile_pool(name="p_bias", bufs=3))
        p_eo = ep(tc.tile_pool(name="p_eo", bufs=4))
        p_wqkv = ep(tc.tile_pool(name="p_wqkv", bufs=1))
        p_wout = ep(tc.tile_pool(name="p_wout", bufs=2))
        p_wca = ep(tc.tile_pool(name="p_wca", bufs=1))
        p_kvx = ep(tc.tile_pool(name="p_kvx", bufs=2))
        p_kvp = ep(tc.tile_pool(name="p_kvp", bufs=1))
        p_hx = ep(tc.tile_pool(name="p_hx", bufs=2))
        p_rx = ep(tc.tile_pool(name="p_rx", bufs=1))

        rx_eob = p_rx.tile([P, NDT * TOWN], bf16, tag="rxeob", name="rxeob")
        ebx = pc.tile([P, NDT * TOWN], bf16)

        FP8 = bool(os.environ.get("KQ_FP8"))  # fp8 wire not HW-validated; default bf16
        f8 = mybir.dt.float8e4

        def sa_block(xs_or_ys, wqkv, bias_ap, bias_bf_ap, mode,
                     post_fire=None, pre_attn=None):
            hx = p_hx.tile([P, NDT, TOWN], bf16, tag="hx", name="hx")
            hs = layer_norm(
                xs_or_ys, bf16, None, None,
                out_aps=[hx[:, dt, :] for dt in range(NDT)],
            )
            if FP8:
                # ship h across the pair as fp8 (|h| <~ 6 fits e4m3): halves
                # the AllGather payload (opt-in); peer K/V reprojection runs in bf16
                # from an upcast copy
                h8 = p_rx.tile([P, NDT, TOWN], f8, tag="h8", name="h8")
                for dt in range(NDT):
                    nc.vector.tensor_copy(h8[:, dt, :], hs[dt])
                bout = exchange_fire(
                    h8[:, :, :].rearrange("p a b -> p (a b)"), NDT * TOWN, f8)
            else:
                bout = exchange_fire(
                    hx[:, :, :].rearrange("p a b -> p (a b)"), NDT * TOWN, bf16)
            if post_fire is not None:
                post_fire()
            kvx = p_kvx.tile([P, KV_W], bf16, tag="kvx", name="kvx")
            proj_kv(wqkv, D, 2 * D, D, 2 * D, hs, bias_ap, bias_bf_ap, kvx)
            Qs = proj_fm(wqkv, 0, 4, hs, bias_ap, p_q, "q")
            if pre_attn is not None:
                pre_attn()
            rxh = p_hx.tile([P, NDT, TOWN], bf16, tag="rxh", name="rxh")
            if FP8:
                rx8 = p_rx.tile([P, NDT, TOWN], f8, tag="rx8", name="rx8")
                exchange_recv(bout, rx8[:, :, :].rearrange("p a b -> p (a b)"))
                for dt in range(NDT):
                    nc.vector.tensor_copy(rxh[:, dt, :], rx8[:, dt, :])
            else:
                exchange_recv(bout, rxh[:, :, :].rearrange("p a b -> p (a b)"))
            hsp = [rxh[:, dt, :] for dt in range(NDT)]
            kvp = p_kvp.tile([P, KV_W], bf16, tag="kvp", name="kvp")
            proj_kv(wqkv, D, 2 * D, D, 2 * D, hsp, bias_ap, bias_bf_ap, kvp)
            if DUMP and not dump_done[0]:
                dump_done[0] = True
                nc.sync.dma_start(out=d_hx[:, :], in_=hx[:, :, :].rearrange("p a b -> p (a b)"))
                nc.sync.dma_start(out=d_rxh[:, :], in_=rxh[:, :, :].rearrange("p a b -> p (a b)"))
                nc.sync.dma_start(out=d_kvx[:, :], in_=kvx[:, :])
                nc.sync.dma_start(out=d_kvp[:, :], in_=kvp[:, :])
            return attention(Qs, (kvx, kvp), mode)

        # ================= encoder =================
        xs = []
        for dt in range(NDT):
            x = p_x.tile([P, TOWN], f32, tag="x")
            nc.sync.dma_start(out=x, in_=x0T[dt])
            xs.append(x)

        for l in range(n_enc):
            wqkv = load_w(p_wqkv, ew_qkv[l], NDT, 3 * D, "wqkv")
            wout = load_w(p_wout, ew_out[l], NDT, D, "wout")

            OTs = sa_block(xs, wqkv, eb_qkv[l], eb_qkv_bf[l], "knn",
                           pre_attn=build_mask_hook if l == 0 else None)
            if DUMP and l == 0:
                for kt in range(NKT):
                    nc.sync.dma_start(out=d_allow[kt], in_=allow_sb[kt])
            if l == 0:
                # created after the mask-phase scoped pools release their zone
                p_g = ep(tc.tile_pool(name="p_g", bufs=16))
                p_w1 = ep(tc.tile_pool(name="p_w1", bufs=3))
                p_w2 = ep(tc.tile_pool(name="p_w2", bufs=2))
                p_ckv = ep(tc.tile_pool(name="p_ckv", bufs=2))
            xs = proj_residual(wout, 0, NDT, OTs, eb_out[l], xs)
            if DUMP and l == 0:
                for dt in range(NDT):
                    nc.sync.dma_start(out=d_x1[dt], in_=xs[dt])
            hs = layer_norm(xs, bf16, p_h, "h")
            xs = ffn(ew_f1[l], ew_f2[l], eb_f1[l], eb_f2[l], hs, xs)

        eof = layer_norm(xs, f32, p_eo, "eof")
        for dt in range(NDT):
            nc.sync.dma_start(out=enc_part[dt], in_=eof[dt])
            nc.vector.tensor_copy(ebx[:, dt * TOWN:(dt + 1) * TOWN], eof[dt])
        eob_state = {}

        def eob_fire_hook():
            eob_state["bout"] = exchange_fire(ebx[:, :], NDT * TOWN, bf16)

        def eob_recv_hook():
            if "bout" in eob_state:
                exchange_recv(eob_state.pop("bout"), rx_eob[:, :])
        eob_own = [ebx[:, dt * TOWN:(dt + 1) * TOWN] for dt in range(NDT)]
        eob_peer = [rx_eob[:, dt * TOWN:(dt + 1) * TOWN] for dt in range(NDT)]

        # ================= decoder =================
        ys = []
        for dt in range(NDT):
            y = p_x.tile([P, TOWN], f32, tag="x")
            nc.sync.dma_start(out=y, in_=y0T[dt])
            ys.append(y)

        for l in range(n_dec):
            wqkv = load_w(p_wqkv, dw_saqkv[l], NDT, 3 * D, "wqkv")
            wout = load_w(p_wout, dw_saout[l], NDT, D, "wout")

            # CA K/V (independent of SA) emitted inside the SA block right
            # after the exchange fires -- fills the PE during the AG wait
            ca_state = {}

            def ca_proj_hook(l=l):
                eob_recv_hook()
                wcakv = p_wca.tile([P, NDT, 2 * D], bf16, tag="wcakv", name="wcakv")
                nc.sync.dma_start(
                    out=wcakv,
                    in_=dw_caqkv[l][:, D:3 * D].rearrange("(kc p) m -> p kc m", p=P),
                )
                ca_own = p_ckv.tile([P, KV_W], bf16, tag="cakv", name="cakv")
                ca_peer = p_ckv.tile([P, KV_W], bf16, tag="cakv", name="cakv")
                proj_kv(wcakv, 0, D, D, 2 * D, eob_own, db_caqkv[l], db_caqkv_bf[l], ca_own)
                proj_kv(wcakv, 0, D, D, 2 * D, eob_peer, db_caqkv[l], db_caqkv_bf[l], ca_peer)
                ca_state["kv"] = (ca_own, ca_peer)

            def dec0_hook():
                eob_fire_hook()

            # self-attention (causal)
            OTs = sa_block(ys, wqkv, db_saqkv[l], db_saqkv_bf[l], "causal",
                           post_fire=eob_fire_hook if l == 0 else None,
                           pre_attn=None if l == 0 else ca_proj_hook)
            if l == 0:
                ca_proj_hook()
            ys = proj_residual(wout, 0, NDT, OTs, db_saout[l], ys)

            ca_own, ca_peer = ca_state["kv"]
            wcaq = load_w(p_wout, dw_caqkv[l][:, 0:D], NDT, D, "wout")
            wcao = load_w(p_wout, dw_caout[l], NDT, D, "wout")
            hs = layer_norm(ys, bf16, p_h, "h")
            Qs = proj_fm(wcaq, 0, 4, hs, db_caqkv[l], p_q, "q")
            OTs = attention(Qs, (ca_own, ca_peer), None)
            ys = proj_residual(wcao, 0, NDT, OTs, db_caout[l], ys)

            # ffn
            hs = layer_norm(ys, bf16, p_h, "h")
            ys = ffn(dw_f1[l], dw_f2[l], db_f1[l], db_f2[l], hs, ys)

        dof = layer_norm(ys, f32, p_eo, "eof")
        for dt in range(NDT):
            nc.sync.dma_start(out=dec_part[dt], in_=dof[dt])

    nc.compile()
    return nc


def make_in_maps(inputs):
    inp = {k: np.asarray(v) for k, v in inputs.items()}
    f32 = np.float32

    W = {
        "ew_qkv": np.ascontiguousarray(inp["e_qkv_w"].swapaxes(1, 2)).astype(BF16),
        "ew_out": np.ascontiguousarray(inp["e_out_w"].swapaxes(1, 2)).astype(BF16),
        "ew_f1": np.ascontiguousarray(inp["e_ff1_w"].swapaxes(1, 2)).astype(BF16),
        "ew_f2": np.ascontiguousarray(inp["e_ff2_w"].swapaxes(1, 2)).astype(BF16),
        "eb_qkv": inp["e_qkv_b"].astype(f32).reshape(NE, 3 * D, 1),
        "eb_out": (inp["e_out_b"] + np.einsum(
            "lmd,ld->lm", inp["e_out_w"], inp["e_qkv_b"][:, 2 * D:])
        ).astype(f32).reshape(NE, D, 1),
        "eb_f1": inp["e_ff1_b"].astype(f32).reshape(NE, F, 1),
        "eb_f2": inp["e_ff2_b"].astype(f32).reshape(NE, D, 1),
        "dw_saqkv": np.ascontiguousarray(inp["d_sa_qkv_w"].swapaxes(1, 2)).astype(BF16),
        "db_saqkv": inp["d_sa_qkv_b"].astype(f32).reshape(ND, 3 * D, 1),
        "dw_saout": np.ascontiguousarray(inp["d_sa_out_w"].swapaxes(1, 2)).astype(BF16),
        "db_saout": (inp["d_sa_out_b"] + np.einsum(
            "lmd,ld->lm", inp["d_sa_out_w"], inp["d_sa_qkv_b"][:, 2 * D:])
        ).astype(f32).reshape(ND, D, 1),
        "dw_caqkv": np.ascontiguousarray(inp["d_ca_qkv_w"].swapaxes(1, 2)).astype(BF16),
        "db_caqkv": inp["d_ca_qkv_b"].astype(f32).reshape(ND, 3 * D, 1),
        "dw_caout": np.ascontiguousarray(inp["d_ca_out_w"].swapaxes(1, 2)).astype(BF16),
        "db_caout": (inp["d_ca_out_b"] + np.einsum(
            "lmd,ld->lm", inp["d_ca_out_w"], inp["d_ca_qkv_b"][:, 2 * D:])
        ).astype(f32).reshape(ND, D, 1),
        "dw_f1": np.ascontiguousarray(inp["d_ff1_w"].swapaxes(1, 2)).astype(BF16),
        "db_f1": inp["d_ff1_b"].astype(f32).reshape(ND, F, 1),
        "dw_f2": np.ascontiguousarray(inp["d_ff2_w"].swapaxes(1, 2)).astype(BF16),
        "db_f2": inp["d_ff2_b"].astype(f32).reshape(ND, D, 1),
        "eb_qkv_bf": inp["e_qkv_b"].astype(BF16).reshape(NE, 3 * D, 1),
        "db_saqkv_bf": inp["d_sa_qkv_b"].astype(BF16).reshape(ND, 3 * D, 1),
        "db_caqkv_bf": inp["d_ca_qkv_b"].astype(BF16).reshape(ND, 3 * D, 1),
    }
    tri = (np.arange(P)[:, None] <= np.arange(P)[None, :]).astype(BF16)

    in_maps = []
    for c in range(NCORE):
        b, half = c // 2, c % 2
        sl = slice(half * TOWN, (half + 1) * TOWN)
        psl = slice((1 - half) * TOWN, (2 - half) * TOWN)
        m = dict(W)
        xT = np.ascontiguousarray(inp["enc_in"][b].astype(f32).T[:, sl])
        m["x0T"] = xT.reshape(NDT, P, TOWN)
        yT = np.ascontiguousarray(inp["dec_in"][b].astype(f32).T[:, sl])
        m["y0T"] = yT.reshape(NDT, P, TOWN)
        xyz = inp["enc_xyz"][b].astype(f32)
        n2 = (xyz * xyz).sum(-1, dtype=f32).astype(f32)
        xq2 = (np.float32(2.0) * xyz[sl]).astype(f32)
        m["xq2"] = np.ascontiguousarray(xq2)
        m["xq2row"] = np.ascontiguousarray(xq2.T)
        xkn = np.concatenate([xyz, n2[:, None]], 1).astype(f32)
        xkn_perm = np.concatenate([xkn[sl], xkn[psl]], 0)  # own-first key order
        m["xkn"] = np.ascontiguousarray(xkn_perm)
        m["xrow"] = np.ascontiguousarray(xkn_perm.T)
        bos = np.full((1, TOWN), 1e30, f32)
        if half == 0:
            bos[0, 0] = NEG
        m["bosrow"] = bos
        boskey = np.zeros((P, NKT), f32)
        boskey[0, 0 if half == 0 else 4] = 1.0
        m["boskey"] = boskey
        m["tri"] = tri
        m["pbias"] = np.full((P, 1), -60.0 if half == 0 else 0.0, f32)
        m["peeridx"] = np.array([[1 - half]], np.uint32)
        in_maps.append(m)
    return in_maps


def assemble(results):
    enc = np.zeros((B, LE, D), np.float32)
    dec = np.zeros((B, LD, D), np.float32)
    for c, r in enumerate(results):
        b, half = c // 2, c % 2
        sl = slice(half * TOWN, (half + 1) * TOWN)
        enc[b, sl, :] = r["enc_part"].reshape(D, TOWN).T
        dec[b, sl, :] = r["dec_part"].reshape(D, TOWN).T
    return enc, dec


def kernel(**inputs):
    from concourse import bass_utils

    if "nc" not in _CACHE:
        _CACHE["nc"] = build()
    nc = _CACHE["nc"]
    in_maps = make_in_maps(inputs)
    res = bass_utils.run_bass_kernel_spmd(
        nc, in_maps, core_ids=list(range(NCORE))
    )
    return assemble(res.results)
